# revision 4
# baseline (speedup 1.0000x reference)
"""CenterLoss forward on 8 Trainium2 NeuronCores (Bass/Tile).

loss = mean_b ||features[b] - centers[labels[b]]||^2  (LAMBDA_C = 1.0)

Strategy (data-parallel, per the sharding hint):
  - Shard features/labels along batch across 8 cores (8192 rows each);
    replicate centers in every core's HBM.
  - Per core: stream the features shard ([128 partitions x 64 rows] layout,
    64KB contiguous per partition), gather the 8192 label-indexed center rows
    with SWDGE indirect DMA (labels live in SBUF as [128, 64] int32), then
    DVE subtract + ACT square-accumulate per chunk, PE ones-matmul for the
    partition reduction, and DMA out a [1,1] partial sum.
  - Host sums the 8 partials and divides by the full batch (the scalar
    "all-reduce" of the mean).
"""

import numpy as np

import concourse.bacc as bacc
import concourse.bass as bass
import concourse.mybir as mybir
import concourse.tile as tile
from concourse.bass_utils import run_bass_kernel_spmd

NCORES = 8
BATCH = 65536
FEAT_DIM = 256
NUM_CLASSES = 100000
LAMBDA_C = 1.0

SHARD = BATCH // NCORES  # 8192 rows per core
P = 128  # SBUF partitions
G = SHARD // P  # 64 examples per partition
FREE = G * FEAT_DIM  # 16384 f32 per partition (64KB)
NCH = 8  # pipeline chunks
GC = G // NCH  # 8 gathered rows per partition per chunk
CFREE = FREE // NCH  # 2048 f32 per partition per chunk

_f32 = mybir.dt.float32


def _build():
    nc = bacc.Bacc(
        "TRN2", target_bir_lowering=False, debug=False, num_devices=NCORES
    )
    feat_d = nc.dram_tensor("features", [SHARD, FEAT_DIM], _f32, kind="ExternalInput")
    lab_d = nc.dram_tensor("labels", [SHARD], mybir.dt.int32, kind="ExternalInput")
    cent_d = nc.dram_tensor(
        "centers", [NUM_CLASSES, FEAT_DIM], _f32, kind="ExternalInput"
    )
    out_d = nc.dram_tensor("partial", [1, 1], _f32, kind="ExternalOutput")

    feat_src = feat_d.ap().rearrange("(p g) d -> p (g d)", p=P)
    lab_src = lab_d.ap().rearrange("(p g) -> p g", p=P)

    with tile.TileContext(nc) as tc:
        with (
            tc.tile_pool(name="big", bufs=1) as big,
            tc.tile_pool(name="sc", bufs=3) as sc,
            tc.tile_pool(name="ps", bufs=1, space="PSUM") as ps,
        ):
            lab = big.tile([P, G], mybir.dt.int32)
            nc.sync.dma_start(out=lab[:], in_=lab_src)

            ones = big.tile([P, 1], _f32)
            nc.vector.memset(ones[:], 1.0)

            feat = big.tile([P, FREE], _f32)
            cent = big.tile([P, FREE], _f32)
            acc = big.tile([P, NCH], _f32)

            for j in range(NCH):
                sl = slice(j * CFREE, (j + 1) * CFREE)
                nc.sync.dma_start(out=feat[:, sl], in_=feat_src[:, sl])
                nc.gpsimd.indirect_dma_start(
                    out=cent[:, sl],
                    out_offset=None,
                    in_=cent_d.ap(),
                    in_offset=bass.IndirectOffsetOnAxis(
                        ap=lab[:, j * GC : (j + 1) * GC], axis=0
                    ),
                )

            for j in range(NCH):
                sl = slice(j * CFREE, (j + 1) * CFREE)
                diff = sc.tile([P, CFREE], _f32, tag="diff")
                nc.vector.tensor_tensor(
                    out=diff[:],
                    in0=feat[:, sl],
                    in1=cent[:, sl],
                    op=mybir.AluOpType.subtract,
                )
                # Square on DVE (exact fp32 multiply; ACT's Square is a
                # piecewise-polynomial approximation on HW), then accumulate
                # along the free dim with ACT's Copy accumulator.
                sq = sc.tile([P, CFREE], _f32, tag="sq")
                nc.vector.tensor_tensor(
                    out=sq[:], in0=diff[:], in1=diff[:], op=mybir.AluOpType.mult
                )
                sqc = sc.tile([P, CFREE], _f32, tag="sqc")
                nc.scalar.activation(
                    out=sqc[:],
                    in_=sq[:],
                    func=mybir.ActivationFunctionType.Copy,
                    accum_out=acc[:, j : j + 1],
                )

            tot = big.tile([P, 1], _f32)
            nc.vector.reduce_sum(out=tot[:], in_=acc[:], axis=mybir.AxisListType.X)

            res_ps = ps.tile([1, 1], _f32)
            nc.tensor.matmul(
                out=res_ps[:], lhsT=ones[:], rhs=tot[:], start=True, stop=True
            )
            res_sb = big.tile([1, 1], _f32)
            nc.vector.tensor_copy(out=res_sb[:], in_=res_ps[:])
            nc.sync.dma_start(out=out_d.ap(), in_=res_sb[:])

    nc.compile()
    return nc


_nc_cache = None


def _get_nc():
    global _nc_cache
    if _nc_cache is None:
        _nc_cache = _build()
    return _nc_cache


def _make_in_maps(features, labels, centers):
    features = np.ascontiguousarray(np.asarray(features, dtype=np.float32))
    labels = np.ascontiguousarray(np.asarray(labels).astype(np.int32))
    centers = np.ascontiguousarray(np.asarray(centers, dtype=np.float32))
    assert features.shape == (BATCH, FEAT_DIM)
    assert labels.shape == (BATCH,)
    assert centers.shape == (NUM_CLASSES, FEAT_DIM)
    return [
        {
            "features": features[k * SHARD : (k + 1) * SHARD],
            "labels": labels[k * SHARD : (k + 1) * SHARD],
            "centers": centers,
        }
        for k in range(NCORES)
    ]


def _reduce_results(results):
    total = sum(float(r["partial"][0, 0]) for r in results)
    return np.float32(LAMBDA_C * total / BATCH)


def kernel(features: np.ndarray, labels: np.ndarray, centers: np.ndarray):
    in_maps = _make_in_maps(features, labels, centers)
    res = run_bass_kernel_spmd(_get_nc(), in_maps, core_ids=list(range(NCORES)))
    return _reduce_results(res.results)


# revision 6
# speedup vs baseline: 1.0416x; 1.0416x over previous
"""CenterLoss forward on 8 Trainium2 NeuronCores (Bass/Tile).

loss = mean_b ||features[b] - centers[labels[b]]||^2  (LAMBDA_C = 1.0)

Strategy (data-parallel, per the sharding hint):
  - Shard features/labels along batch across 8 cores (8192 rows each);
    replicate centers in every core's HBM.
  - Per core: stream the features shard ([128 partitions x 64 rows] layout,
    64KB contiguous per partition), gather the 8192 label-indexed center rows
    with SWDGE indirect DMA (labels live in SBUF as [128, 64] int32), then
    DVE subtract + ACT square-accumulate per chunk, PE ones-matmul for the
    partition reduction, and DMA out a [1,1] partial sum.
  - Host sums the 8 partials and divides by the full batch (the scalar
    "all-reduce" of the mean).
"""

import numpy as np

import concourse.bacc as bacc
import concourse.bass as bass
import concourse.mybir as mybir
import concourse.tile as tile
from concourse.bass_utils import run_bass_kernel_spmd

NCORES = 8
BATCH = 65536
FEAT_DIM = 256
NUM_CLASSES = 100000
LAMBDA_C = 1.0

SHARD = BATCH // NCORES  # 8192 rows per core
P = 128  # SBUF partitions
G = SHARD // P  # 64 examples per partition
FREE = G * FEAT_DIM  # 16384 f32 per partition (64KB)
NCH = 8  # pipeline chunks
GC = G // NCH  # 8 gathered rows per partition per chunk
CFREE = FREE // NCH  # 2048 f32 per partition per chunk

_f32 = mybir.dt.float32


def _build():
    nc = bacc.Bacc(
        "TRN2", target_bir_lowering=False, debug=False, num_devices=NCORES
    )
    feat_d = nc.dram_tensor("features", [SHARD, FEAT_DIM], _f32, kind="ExternalInput")
    lab_d = nc.dram_tensor("labels", [SHARD], mybir.dt.int32, kind="ExternalInput")
    cent_d = nc.dram_tensor(
        "centers", [NUM_CLASSES, FEAT_DIM], _f32, kind="ExternalInput"
    )
    out_d = nc.dram_tensor("partial", [1, 1], _f32, kind="ExternalOutput")

    feat_src = feat_d.ap().rearrange("(p g) d -> p (g d)", p=P)
    lab_src = lab_d.ap().rearrange("(p g) -> p g", p=P)

    with tile.TileContext(nc) as tc:
        with (
            tc.tile_pool(name="big", bufs=1) as big,
            tc.tile_pool(name="sc", bufs=3) as sc,
            tc.tile_pool(name="ps", bufs=1, space="PSUM") as ps,
        ):
            lab = big.tile([P, G], mybir.dt.int32)
            nc.sync.dma_start(out=lab[:], in_=lab_src)

            ones = big.tile([P, 1], _f32)
            nc.vector.memset(ones[:], 1.0)

            feat = big.tile([P, FREE], _f32)
            cent = big.tile([P, FREE], _f32)

            for j in range(NCH):
                sl = slice(j * CFREE, (j + 1) * CFREE)
                nc.sync.dma_start(out=feat[:, sl], in_=feat_src[:, sl])
                nc.gpsimd.indirect_dma_start(
                    out=cent[:, sl],
                    out_offset=None,
                    in_=cent_d.ap(),
                    in_offset=bass.IndirectOffsetOnAxis(
                        ap=lab[:, j * GC : (j + 1) * GC], axis=0
                    ),
                )

            # PSUM accumulator row: res_ps[0, n] accumulates
            # sum_p sq[p, k*512 + n] across every chunk via ones-matmuls.
            # PSUM adders are plain fp32 (the ACT accum_out accumulator
            # rounds with bias on HW — measured +2.9e-4 on this sum).
            MMN = 512
            nmm = CFREE // MMN
            res_ps = ps.tile([1, MMN], _f32)
            mm = 0
            for j in range(NCH):
                sl = slice(j * CFREE, (j + 1) * CFREE)
                diff = sc.tile([P, CFREE], _f32, tag="diff")
                nc.vector.tensor_tensor(
                    out=diff[:],
                    in0=feat[:, sl],
                    in1=cent[:, sl],
                    op=mybir.AluOpType.subtract,
                )
                # Square on DVE (exact fp32 multiply; ACT's Square is a
                # piecewise-polynomial approximation on HW).
                sq = sc.tile([P, CFREE], _f32, tag="sq")
                nc.vector.tensor_tensor(
                    out=sq[:], in0=diff[:], in1=diff[:], op=mybir.AluOpType.mult
                )
                for k in range(nmm):
                    nc.tensor.matmul(
                        out=res_ps[:],
                        lhsT=ones[:],
                        rhs=sq[:, k * MMN : (k + 1) * MMN],
                        start=(mm == 0),
                        stop=(mm == NCH * nmm - 1),
                    )
                    mm += 1

            row = big.tile([1, MMN], _f32)
            nc.vector.tensor_copy(out=row[:], in_=res_ps[:])
            res_sb = big.tile([1, 1], _f32)
            nc.vector.reduce_sum(
                out=res_sb[:], in_=row[:], axis=mybir.AxisListType.X
            )
            nc.sync.dma_start(out=out_d.ap(), in_=res_sb[:])

    nc.compile()
    return nc


_nc_cache = None


def _get_nc():
    global _nc_cache
    if _nc_cache is None:
        _nc_cache = _build()
    return _nc_cache


def _make_in_maps(features, labels, centers):
    features = np.ascontiguousarray(np.asarray(features, dtype=np.float32))
    labels = np.ascontiguousarray(np.asarray(labels).astype(np.int32))
    centers = np.ascontiguousarray(np.asarray(centers, dtype=np.float32))
    assert features.shape == (BATCH, FEAT_DIM)
    assert labels.shape == (BATCH,)
    assert centers.shape == (NUM_CLASSES, FEAT_DIM)
    return [
        {
            "features": features[k * SHARD : (k + 1) * SHARD],
            "labels": labels[k * SHARD : (k + 1) * SHARD],
            "centers": centers,
        }
        for k in range(NCORES)
    ]


def _reduce_results(results):
    total = sum(float(r["partial"][0, 0]) for r in results)
    return np.float32(LAMBDA_C * total / BATCH)


def kernel(features: np.ndarray, labels: np.ndarray, centers: np.ndarray):
    in_maps = _make_in_maps(features, labels, centers)
    res = run_bass_kernel_spmd(_get_nc(), in_maps, core_ids=list(range(NCORES)))
    return _reduce_results(res.results)
